# revision 12
# baseline (speedup 1.0000x reference)
"""Fused QKV-projection + multi-head attention on 8 TRN2 NeuronCores.

Problem (dense_transformer): B=4, S=2048, H=16, D=128, E=2048.
  qkv = x @ W.T + b ; q,k,v per head ; y = softmax(q k^T / sqrt(D)) v
Returns (y [B*S, E], k [B, H, S, D], v [B, H, S, D]) like the reference.

Sharding: 8 cores = 4 batches x 2 head-groups (8 heads each). Outputs are
fully disjoint across cores -> zero collectives; host gathers.

Per-core kernel (SPMD, identical program):
  Phase 1 (f32r matmuls): xT kept resident in SBUF.
    q,k in [o, s] orientation (bias = per-partition via K=1 matmul b x ones)
    v   in [s, o] natural orientation (bias via K=1 matmul ones x b)
  Phase 2 per head: scoresT[j,i] = kT^T q (f32r) -> ACT exp (scale fused)
    -> bf16 expT -> AV matmuls with expT slices stationary and a
    ones-augmented V (moving [j, 129]) so softmax row-sums accumulate in
    PSUM column 128 -> per-partition reciprocal * y -> natural-layout y.
"""

import math
from functools import lru_cache

import numpy as np

import concourse.bass as bass
import concourse.tile as tile
from concourse import mybir
import concourse.bass_utils as bass_utils

P = 128          # partitions
SEQ = 2048       # sequence length (per batch)
E = 2048         # embedding
D = 128          # head dim
NHL = 8          # heads per core
EC = E // P      # e-chunks
NJC = SEQ // P   # j-chunks (keys)
NIC = SEQ // P   # i-chunks (queries)
SCALE = 1.0 / math.sqrt(D)

F32 = mybir.dt.float32
F32R = mybir.dt.float32r
BF16 = mybir.dt.bfloat16

# Most production walrus builds accept a limited number of inline sync waits
# per engine instruction (1 here). Hoist extras onto same-engine NoOps; DMAs
# here are SP-sequencer-issued (PSEUDO_DMA_DIRECT2D), so a preceding SP NoOp
# gives identical ordering.
_NO_SPLIT = ()


def _split_multiwaits(nc):
    for f in nc.m.functions:
        for blk in f.blocks:
            insts = list(blk.instructions)
            new = []
            changed = False
            for inst in insts:
                si = inst.sync_info
                if (
                    si is not None
                    and len(si.on_wait) > 1
                    and type(inst).__name__ not in _NO_SPLIT
                ):
                    waits = list(si.on_wait)
                    for w in waits[:-1]:
                        new.append(
                            mybir.InstNoOp(
                                name=nc.get_next_instruction_name(),
                                engine=inst.engine,
                                sync_info=mybir.SyncInfo(on_wait=[w], on_update=[]),
                                bass_nofuse=True,
                            )
                        )
                    inst.sync_info = mybir.SyncInfo(
                        on_wait=[waits[-1]], on_update=list(si.on_update)
                    )
                    changed = True
                new.append(inst)
            if changed:
                blk.instructions = new


def _patch_upload():
    orig = bass_utils.upload_artifacts
    if getattr(orig, "_safe_wrapped", False):
        return

    def safe_upload(tmpdir):
        try:
            return orig(tmpdir)
        except Exception:
            return tmpdir

    safe_upload._safe_wrapped = True
    bass_utils.upload_artifacts = safe_upload


@lru_cache(maxsize=1)
def _build():
    nc = bass.Bass()

    xT = nc.declare_dram_parameter("xT", [E, SEQ], F32R, isOutput=False)
    wT = nc.declare_dram_parameter("wT", [E, 3 * NHL * D], F32R, isOutput=False)
    bias = nc.declare_dram_parameter("bias", [1, 3 * NHL * D], F32R, isOutput=False)
    kT_out = nc.declare_dram_parameter("kT_out", [NHL * D, SEQ], F32R, isOutput=True)
    v_out = nc.declare_dram_parameter("v_out", [SEQ, NHL * D], F32R, isOutput=True)
    y_out = nc.declare_dram_parameter("y_out", [SEQ, NHL * D], F32, isOutput=True)
    qT_d = nc.dram_tensor("qT_d", [NHL * D, SEQ], F32R)

    OQ = NHL * D          # 1024: q columns [0, OQ), k [OQ, 2*OQ), v [2*OQ, 3*OQ)

    with tile.TileContext(nc) as tc:
        with tc.tile_pool(name="const", bufs=1) as const_pool:
            b_sb = const_pool.tile([1, 3 * OQ], F32R)
            nc.sync.dma_start(out=b_sb[:], in_=bias[:])
            ones_f32 = const_pool.tile([1, 512], F32)
            nc.vector.memset(ones_f32[:], 1.0)
            ones_sb = const_pool.tile([1, 512], F32R)
            nc.vector.tensor_copy(ones_sb[:], ones_f32[:])

            # ---------------- Phase 1: QKV projection ----------------
            with (
                tc.tile_pool(name="xr", bufs=1) as xr_pool,
                tc.tile_pool(name="qk_ps", bufs=8, space="PSUM") as qk_ps,
            ):
                # q,k part: [o, s] tiles; o-chunk 0..7 = q heads, 8..15 = k heads
                with (
                    tc.tile_pool(name="wqk", bufs=3) as wqk_pool,
                    tc.tile_pool(name="kqst", bufs=3) as kq_stage_pool,
                ):
                    # First two w tiles BEFORE the bulk xT load so oc=0/1
                    # matmuls can start as soon as early xT e-chunks land.
                    w_tiles = {}
                    for oc in range(2):
                        w_t = wqk_pool.tile(
                            [P, EC, P], F32R, tag="wqk", name=f"wpre{oc}"
                        )
                        nc.sync.dma_start(
                            out=w_t[:],
                            in_=wT[:, oc * P : (oc + 1) * P].rearrange(
                                "(ec p) o -> p ec o", p=P
                            ),
                        )
                        w_tiles[oc] = w_t

                    xr = []
                    for ec in range(EC):
                        t = xr_pool.tile([P, SEQ], F32R, tag=f"xr{ec}")
                        nc.sync.dma_start(out=t[:], in_=xT[ec * P : (ec + 1) * P, :])
                        xr.append(t)

                    for oc in range(16):
                        if oc in w_tiles:
                            w_t = w_tiles.pop(oc)
                        else:
                            w_t = wqk_pool.tile([P, EC, P], F32R, tag="wqk")
                            nc.sync.dma_start(
                                out=w_t[:],
                                in_=wT[:, oc * P : (oc + 1) * P].rearrange(
                                    "(ec p) o -> p ec o", p=P
                                ),
                            )
                        kq_stage = kq_stage_pool.tile([P, SEQ], F32R, tag="kq")
                        # ec-outer: each stationary (bias slice / w e-chunk) is
                        # loaded once and reused across the 4 s-chunks.
                        pss = [qk_ps.tile([P, 512], F32, tag="pqk", name=f"pqk{oc}_{i}") for i in range(4)]
                        for s4 in range(4):
                            nc.tensor.matmul(
                                pss[s4][:],
                                b_sb[0:1, oc * P : (oc + 1) * P],
                                ones_sb[0:1, :],
                                start=True,
                                stop=False,
                            )
                        for ec in range(EC):
                            for s4 in range(4):
                                nc.tensor.matmul(
                                    pss[s4][:],
                                    w_t[:, ec, :],
                                    xr[ec][:, s4 * 512 : (s4 + 1) * 512],
                                    start=False,
                                    stop=(ec == EC - 1),
                                )
                        for s4 in range(4):
                            nc.vector.tensor_copy(
                                kq_stage[:, s4 * 512 : (s4 + 1) * 512], pss[s4][:]
                            )
                        if oc < NHL:
                            nc.sync.dma_start(
                                out=qT_d[oc * P : (oc + 1) * P, :], in_=kq_stage[:]
                            )
                        else:
                            nc.sync.dma_start(
                                out=kT_out[(oc - NHL) * P : (oc - NHL + 1) * P, :],
                                in_=kq_stage[:],
                            )

                # v part: natural [s, o] tiles
                with (
                    tc.tile_pool(name="wv", bufs=2) as wv_pool,
                    tc.tile_pool(name="vst", bufs=3) as v_stage_pool,
                ):
                    VW = 256
                    for oh in range(1024 // VW):
                        wv_t = wv_pool.tile([P, EC, VW], F32R, tag="wv")
                        nc.sync.dma_start(
                            out=wv_t[:],
                            in_=wT[
                                :, 2 * OQ + oh * VW : 2 * OQ + (oh + 1) * VW
                            ].rearrange("(ec p) o -> p ec o", p=P),
                        )
                        for sc in range(NJC):
                            ps_t = qk_ps.tile([P, 512], F32, tag="pqk")
                            nc.tensor.matmul(
                                ps_t[:, 0:VW],
                                ones_sb[0:1, 0:P],
                                b_sb[0:1, 2 * OQ + oh * VW : 2 * OQ + (oh + 1) * VW],
                                start=True,
                                stop=False,
                            )
                            for ec in range(EC):
                                nc.tensor.matmul(
                                    ps_t[:, 0:VW],
                                    xr[ec][:, sc * P : (sc + 1) * P],
                                    wv_t[:, ec, :],
                                    start=False,
                                    stop=(ec == EC - 1),
                                )
                            v_stage = v_stage_pool.tile([P, VW], F32R, tag="vs")
                            nc.vector.tensor_copy(v_stage[:], ps_t[:, 0:VW])
                            nc.sync.dma_start(
                                out=v_out[
                                    sc * P : (sc + 1) * P,
                                    oh * VW : (oh + 1) * VW,
                                ],
                                in_=v_stage[:],
                            )

            # ---------------- Phase 2: attention per head ----------------
            with (
                tc.tile_pool(name="qh", bufs=2) as q_pool,
                tc.tile_pool(name="kh", bufs=2) as k_pool,
                tc.tile_pool(name="vldst", bufs=4) as vld_pool,
                tc.tile_pool(name="vaug", bufs=34) as vaug_pool,
                tc.tile_pool(name="expt", bufs=33) as expt_pool,
                tc.tile_pool(name="yst", bufs=4) as y_stage_pool,
                tc.tile_pool(name="rec", bufs=4) as rec_pool,
                tc.tile_pool(name="sc_ps", bufs=1, space="PSUM") as sc_ps,
                tc.tile_pool(name="y_ps", bufs=2, space="PSUM") as y_ps,
            ):
                # Software pipeline: within head h's scores/exp loop (ACT-bound,
                # ~2us/chunk) the PE dovetails one AV accumulation group of head
                # h-1 per j-chunk, so exp latency hides under AV matmuls.
                prev = None  # (expts, vaugs, h-1)

                def av_group(expts_, vaugs_, h_, ic):
                    ps_y = y_ps.tile([P, D + 1], F32, tag="psy", name=f"psy{h_}_{ic}")
                    for jj in range(NJC):
                        nc.tensor.matmul(
                            ps_y[:],
                            expts_[jj][:, ic * P : (ic + 1) * P],
                            vaugs_[jj][:],
                            start=(jj == 0),
                            stop=(jj == NJC - 1),
                        )
                    rec = rec_pool.tile([P, 1], F32, tag="rc", name=f"rc{h_}_{ic}")
                    nc.vector.reciprocal(rec[:], ps_y[:, D : D + 1])
                    ysb = y_stage_pool.tile([P, D], F32, tag="ys", name=f"ys{h_}_{ic}")
                    nc.vector.tensor_scalar_mul(ysb[:], ps_y[:, 0:D], rec[:])
                    nc.sync.dma_start(
                        out=y_out[ic * P : (ic + 1) * P, h_ * D : (h_ + 1) * D],
                        in_=ysb[:],
                    )

                for h in range(NHL):
                    qr = q_pool.tile([P, SEQ], F32R, tag="q")
                    nc.sync.dma_start(out=qr[:], in_=qT_d[h * P : (h + 1) * P, :])
                    kr = k_pool.tile([P, SEQ], F32R, tag="k")
                    nc.sync.dma_start(out=kr[:], in_=kT_out[h * P : (h + 1) * P, :])

                    vaugs = []
                    for jc in range(NJC):
                        vs = vld_pool.tile([P, D], F32R, tag="vl", name=f"vl{h}_{jc}")
                        nc.sync.dma_start(
                            out=vs[:],
                            in_=v_out[jc * P : (jc + 1) * P, h * D : (h + 1) * D],
                        )
                        va = vaug_pool.tile(
                            [P, D + 1], BF16, tag="va", name=f"va{h}_{jc}"
                        )
                        nc.vector.tensor_copy(va[:, 0:D], vs[:])
                        nc.vector.memset(va[:, D : D + 1], 1.0)
                        vaugs.append(va)

                    expts = []
                    for jc in range(NJC):
                        et = expt_pool.tile([P, SEQ], BF16, tag="et", name=f"et{h}_{jc}")
                        ps_sc = sc_ps.tile([P, SEQ], F32, tag="sc", name=f"sc{h}_{jc}")
                        for i5 in range(4):
                            nc.tensor.matmul(
                                ps_sc[:, i5 * 512 : (i5 + 1) * 512],
                                kr[:, jc * P : (jc + 1) * P],
                                qr[:, i5 * 512 : (i5 + 1) * 512],
                                start=True,
                                stop=True,
                            )
                        nc.scalar.activation(
                            et[:],
                            ps_sc[:],
                            mybir.ActivationFunctionType.Exp,
                            scale=SCALE,
                        )
                        expts.append(et)
                        if prev is not None:
                            av_group(prev[0], prev[1], prev[2], jc)
                    prev = (expts, vaugs, h)

                # drain: AV for the last head
                for ic in range(NIC):
                    av_group(prev[0], prev[1], prev[2], ic)

    _split_multiwaits(nc)
    return nc


def kernel(x, W, b):
    x = np.ascontiguousarray(np.asarray(x, dtype=np.float32))
    W = np.asarray(W, dtype=np.float32)
    b = np.asarray(b, dtype=np.float32)

    _patch_upload()
    nc = _build()

    B, H = 4, 16
    in_maps = []
    for c in range(8):
        bi, g = divmod(c, 2)
        xTc = np.ascontiguousarray(x[bi * SEQ : (bi + 1) * SEQ, :].T)
        cols = np.concatenate(
            [
                np.arange(g * 1024, (g + 1) * 1024),
                np.arange(2048 + g * 1024, 2048 + (g + 1) * 1024),
                np.arange(4096 + g * 1024, 4096 + (g + 1) * 1024),
            ]
        )
        WTc = np.ascontiguousarray(W[cols, :].T)
        bc = np.ascontiguousarray(b[cols]).reshape(1, 3072)
        in_maps.append({"xT": xTc, "wT": WTc, "bias": bc})

    res = bass_utils.run_bass_kernel_spmd(nc, in_maps, core_ids=list(range(8)))
    global LAST_RESULT
    LAST_RESULT = res

    y = np.empty((B * SEQ, E), np.float32)
    k = np.empty((B, H, SEQ, D), np.float32)
    v = np.empty((B, H, SEQ, D), np.float32)
    for c in range(8):
        bi, g = divmod(c, 2)
        r = res.results[c]
        y[bi * SEQ : (bi + 1) * SEQ, g * 1024 : (g + 1) * 1024] = r["y_out"]
        kT = r["kT_out"]
        vv = r["v_out"]
        for hh in range(NHL):
            k[bi, g * NHL + hh] = kT[hh * D : (hh + 1) * D, :].T
            v[bi, g * NHL + hh] = vv[:, hh * D : (hh + 1) * D]
    return (y, k, v)


LAST_RESULT = None


# revision 14
# speedup vs baseline: 1.3037x; 1.3037x over previous
"""Fused QKV-projection + multi-head attention on 8 TRN2 NeuronCores.

Problem (dense_transformer): B=4, S=2048, H=16, D=128, E=2048.
  qkv = x @ W.T + b ; q,k,v per head ; y = softmax(q k^T / sqrt(D)) v
Returns (y [B*S, E], k [B, H, S, D], v [B, H, S, D]) like the reference.

Sharding: 8 cores = 4 batches x 2 head-groups (8 heads each). Outputs are
fully disjoint across cores -> zero collectives; host gathers.

Per-core kernel (SPMD, identical program):
  Phase 1 (f32r matmuls): xT kept resident in SBUF.
    q,k in [o, s] orientation (bias = per-partition via K=1 matmul b x ones)
    v   in [s, o] natural orientation (bias via K=1 matmul ones x b)
  Phase 2 per head: scoresT[j,i] = kT^T q (f32r) -> ACT exp (scale fused)
    -> bf16 expT -> AV matmuls with expT slices stationary and a
    ones-augmented V (moving [j, 129]) so softmax row-sums accumulate in
    PSUM column 128 -> per-partition reciprocal * y -> natural-layout y.
"""

import math
from functools import lru_cache

import numpy as np

import concourse.bass as bass
import concourse.tile as tile
from concourse import mybir
import concourse.bass_utils as bass_utils

P = 128          # partitions
SEQ = 2048       # sequence length (per batch)
E = 2048         # embedding
D = 128          # head dim
NHL = 8          # heads per core
EC = E // P      # e-chunks
NJC = SEQ // P   # j-chunks (keys)
NIC = SEQ // P   # i-chunks (queries)
SCALE = 1.0 / math.sqrt(D)

F32 = mybir.dt.float32
F32R = mybir.dt.float32r
BF16 = mybir.dt.bfloat16

# Most production walrus builds accept a limited number of inline sync waits
# per engine instruction (1 here). Hoist extras onto same-engine NoOps; DMAs
# here are SP-sequencer-issued (PSEUDO_DMA_DIRECT2D), so a preceding SP NoOp
# gives identical ordering.
_NO_SPLIT = ()


def _split_multiwaits(nc):
    for f in nc.m.functions:
        for blk in f.blocks:
            insts = list(blk.instructions)
            new = []
            changed = False
            for inst in insts:
                si = inst.sync_info
                if (
                    si is not None
                    and len(si.on_wait) > 1
                    and type(inst).__name__ not in _NO_SPLIT
                ):
                    waits = list(si.on_wait)
                    for w in waits[:-1]:
                        new.append(
                            mybir.InstNoOp(
                                name=nc.get_next_instruction_name(),
                                engine=inst.engine,
                                sync_info=mybir.SyncInfo(on_wait=[w], on_update=[]),
                                bass_nofuse=True,
                            )
                        )
                    inst.sync_info = mybir.SyncInfo(
                        on_wait=[waits[-1]], on_update=list(si.on_update)
                    )
                    changed = True
                new.append(inst)
            if changed:
                blk.instructions = new


def _patch_upload():
    orig = bass_utils.upload_artifacts
    if getattr(orig, "_safe_wrapped", False):
        return

    def safe_upload(tmpdir):
        try:
            return orig(tmpdir)
        except Exception:
            return tmpdir

    safe_upload._safe_wrapped = True
    bass_utils.upload_artifacts = safe_upload


@lru_cache(maxsize=1)
def _build():
    nc = bass.Bass()

    xT = nc.declare_dram_parameter("xT", [E, SEQ], F32R, isOutput=False)
    wT = nc.declare_dram_parameter("wT", [E, 3 * NHL * D], F32R, isOutput=False)
    bias = nc.declare_dram_parameter("bias", [1, 3 * NHL * D], F32R, isOutput=False)
    biasT = nc.declare_dram_parameter("biasT", [3 * NHL * D, 1], F32R, isOutput=False)
    kT_out = nc.declare_dram_parameter("kT_out", [NHL * D, SEQ], F32R, isOutput=True)
    v_out = nc.declare_dram_parameter("v_out", [SEQ, NHL * D], F32R, isOutput=True)
    y_out = nc.declare_dram_parameter("y_out", [SEQ, NHL * D], F32, isOutput=True)
    qT_d = nc.dram_tensor("qT_d", [NHL * D, SEQ], F32R)

    OQ = NHL * D          # 1024: q columns [0, OQ), k [OQ, 2*OQ), v [2*OQ, 3*OQ)

    with tile.TileContext(nc) as tc:
        with tc.tile_pool(name="const", bufs=1) as const_pool:
            b_sb = const_pool.tile([1, 3 * OQ], F32R)
            nc.sync.dma_start(out=b_sb[:], in_=bias[:])
            ones_f32 = const_pool.tile([1, 512], F32)
            nc.vector.memset(ones_f32[:], 1.0)
            ones_sb = const_pool.tile([1, 512], F32R)
            nc.vector.tensor_copy(ones_sb[:], ones_f32[:])
            bT_t = const_pool.tile([P, 3 * NHL * D // P], F32R)
            nc.sync.dma_start(
                out=bT_t[:], in_=biasT.rearrange("(c p) one -> p (c one)", p=P)
            )

            # ---------------- Phase 1: QKV projection ----------------
            with (
                tc.tile_pool(name="xr", bufs=1) as xr_pool,
                tc.tile_pool(name="qk_ps", bufs=8, space="PSUM") as qk_ps,
            ):
                # q,k part: [o, s] tiles; o-chunk 0..7 = q heads, 8..15 = k heads
                with (
                    tc.tile_pool(name="wqk", bufs=3) as wqk_pool,
                    tc.tile_pool(name="kqst", bufs=3) as kq_stage_pool,
                ):
                    # First two w tiles BEFORE the bulk xT load so oc=0/1
                    # matmuls can start as soon as early xT e-chunks land.
                    w_tiles = {}
                    for oc in range(2):
                        w_t = wqk_pool.tile(
                            [P, EC, P], F32R, tag="wqk", name=f"wpre{oc}"
                        )
                        nc.sync.dma_start(
                            out=w_t[:],
                            in_=wT[:, oc * P : (oc + 1) * P].rearrange(
                                "(ec p) o -> p ec o", p=P
                            ),
                        )
                        w_tiles[oc] = w_t

                    xr = []
                    for ec in range(EC):
                        t = xr_pool.tile([P, SEQ], F32R, tag=f"xr{ec}")
                        nc.sync.dma_start(out=t[:], in_=xT[ec * P : (ec + 1) * P, :])
                        xr.append(t)

                    for oc in range(16):
                        if oc in w_tiles:
                            w_t = w_tiles.pop(oc)
                        else:
                            w_t = wqk_pool.tile([P, EC, P], F32R, tag="wqk")
                            nc.sync.dma_start(
                                out=w_t[:],
                                in_=wT[:, oc * P : (oc + 1) * P].rearrange(
                                    "(ec p) o -> p ec o", p=P
                                ),
                            )
                        kq_stage = kq_stage_pool.tile([P, SEQ], F32R, tag="kq")
                        # ec-outer: each stationary (bias slice / w e-chunk) is
                        # loaded once and reused across the 4 s-chunks.
                        pss = [qk_ps.tile([P, 512], F32, tag="pqk", name=f"pqk{oc}_{i}") for i in range(4)]
                        for ec in range(EC):
                            for s4 in range(4):
                                nc.tensor.matmul(
                                    pss[s4][:],
                                    w_t[:, ec, :],
                                    xr[ec][:, s4 * 512 : (s4 + 1) * 512],
                                    start=(ec == 0),
                                    stop=(ec == EC - 1),
                                )
                        for s4 in range(4):
                            # bias folded into eviction: per-partition add
                            nc.vector.tensor_scalar_add(
                                kq_stage[:, s4 * 512 : (s4 + 1) * 512],
                                pss[s4][:],
                                bT_t[:, oc : oc + 1].bitcast(F32),
                            )
                        if oc < NHL:
                            nc.sync.dma_start(
                                out=qT_d[oc * P : (oc + 1) * P, :], in_=kq_stage[:]
                            )
                        else:
                            nc.sync.dma_start(
                                out=kT_out[(oc - NHL) * P : (oc - NHL + 1) * P, :],
                                in_=kq_stage[:],
                            )

                # v part: natural [s, o] tiles
                with (
                    tc.tile_pool(name="wv", bufs=2) as wv_pool,
                    tc.tile_pool(name="vst", bufs=3) as v_stage_pool,
                ):
                    VW = 256
                    for oh in range(1024 // VW):
                        wv_t = wv_pool.tile([P, EC, VW], F32R, tag="wv")
                        nc.sync.dma_start(
                            out=wv_t[:],
                            in_=wT[
                                :, 2 * OQ + oh * VW : 2 * OQ + (oh + 1) * VW
                            ].rearrange("(ec p) o -> p ec o", p=P),
                        )
                        for sc in range(NJC):
                            ps_t = qk_ps.tile([P, 512], F32, tag="pqk")
                            nc.tensor.matmul(
                                ps_t[:, 0:VW],
                                ones_sb[0:1, 0:P],
                                b_sb[0:1, 2 * OQ + oh * VW : 2 * OQ + (oh + 1) * VW],
                                start=True,
                                stop=False,
                            )
                            for ec in range(EC):
                                nc.tensor.matmul(
                                    ps_t[:, 0:VW],
                                    xr[ec][:, sc * P : (sc + 1) * P],
                                    wv_t[:, ec, :],
                                    start=False,
                                    stop=(ec == EC - 1),
                                )
                            v_stage = v_stage_pool.tile([P, VW], F32R, tag="vs")
                            nc.vector.tensor_copy(v_stage[:], ps_t[:, 0:VW])
                            nc.sync.dma_start(
                                out=v_out[
                                    sc * P : (sc + 1) * P,
                                    oh * VW : (oh + 1) * VW,
                                ],
                                in_=v_stage[:],
                            )

            # ---------------- Phase 2: attention per head ----------------
            with (
                tc.tile_pool(name="qh", bufs=2) as q_pool,
                tc.tile_pool(name="kh", bufs=2) as k_pool,
                tc.tile_pool(name="vldst", bufs=4) as vld_pool,
                tc.tile_pool(name="vaug", bufs=34) as vaug_pool,
                tc.tile_pool(name="expt", bufs=33) as expt_pool,
                tc.tile_pool(name="yst", bufs=4) as y_stage_pool,
                tc.tile_pool(name="rec", bufs=4) as rec_pool,
                tc.tile_pool(name="sc_ps", bufs=3, space="PSUM") as sc_ps,
                tc.tile_pool(name="y_ps", bufs=2, space="PSUM") as y_ps,
            ):
                # Software pipeline: within head h's scores/exp loop (ACT-bound,
                # ~2us/chunk) the PE dovetails one AV accumulation group of head
                # h-1 per j-chunk, so exp latency hides under AV matmuls.
                prev = None  # (expts, vaugs, h-1)

                def av_group(expts_, vaugs_, h_, ic):
                    ps_y = y_ps.tile([P, D + 1], F32, tag="psy", name=f"psy{h_}_{ic}")
                    for jj in range(NJC):
                        nc.tensor.matmul(
                            ps_y[:],
                            expts_[jj][:, ic * P : (ic + 1) * P],
                            vaugs_[jj][:],
                            start=(jj == 0),
                            stop=(jj == NJC - 1),
                        )
                    rec = rec_pool.tile([P, 1], F32, tag="rc", name=f"rc{h_}_{ic}")
                    nc.vector.reciprocal(rec[:], ps_y[:, D : D + 1])
                    ysb = y_stage_pool.tile([P, D], F32, tag="ys", name=f"ys{h_}_{ic}")
                    nc.vector.tensor_scalar_mul(ysb[:], ps_y[:, 0:D], rec[:])
                    nc.sync.dma_start(
                        out=y_out[ic * P : (ic + 1) * P, h_ * D : (h_ + 1) * D],
                        in_=ysb[:],
                    )

                for h in range(NHL):
                    qr = q_pool.tile([P, SEQ], F32R, tag="q")
                    nc.sync.dma_start(out=qr[:], in_=qT_d[h * P : (h + 1) * P, :])
                    kr = k_pool.tile([P, SEQ], F32R, tag="k")
                    nc.sync.dma_start(out=kr[:], in_=kT_out[h * P : (h + 1) * P, :])

                    vaugs = []
                    for jc in range(NJC):
                        vs = vld_pool.tile([P, D], F32R, tag="vl", name=f"vl{h}_{jc}")
                        nc.sync.dma_start(
                            out=vs[:],
                            in_=v_out[jc * P : (jc + 1) * P, h * D : (h + 1) * D],
                        )
                        va = vaug_pool.tile(
                            [P, D + 1], BF16, tag="va", name=f"va{h}_{jc}"
                        )
                        nc.vector.tensor_copy(va[:, 0:D], vs[:])
                        nc.vector.memset(va[:, D : D + 1], 1.0)
                        vaugs.append(va)

                    expts = []
                    for jc in range(NJC):
                        et = expt_pool.tile([P, SEQ], BF16, tag="et", name=f"et{h}_{jc}")
                        for ih in range(2):
                            ps_sc = sc_ps.tile(
                                [P, 1024], F32, tag="sc", name=f"sc{h}_{jc}_{ih}"
                            )
                            for i5 in range(2):
                                nc.tensor.matmul(
                                    ps_sc[:, i5 * 512 : (i5 + 1) * 512],
                                    kr[:, jc * P : (jc + 1) * P],
                                    qr[
                                        :,
                                        ih * 1024 + i5 * 512 : ih * 1024 + (i5 + 1) * 512,
                                    ],
                                    start=True,
                                    stop=True,
                                )
                            nc.scalar.activation(
                                et[:, ih * 1024 : (ih + 1) * 1024],
                                ps_sc[:],
                                mybir.ActivationFunctionType.Exp,
                                scale=SCALE,
                            )
                        expts.append(et)
                        if prev is not None:
                            av_group(prev[0], prev[1], prev[2], jc)
                    prev = (expts, vaugs, h)

                # drain: AV for the last head
                for ic in range(NIC):
                    av_group(prev[0], prev[1], prev[2], ic)

    _split_multiwaits(nc)
    return nc


def kernel(x, W, b):
    x = np.ascontiguousarray(np.asarray(x, dtype=np.float32))
    W = np.asarray(W, dtype=np.float32)
    b = np.asarray(b, dtype=np.float32)

    _patch_upload()
    nc = _build()

    B, H = 4, 16
    in_maps = []
    for c in range(8):
        bi, g = divmod(c, 2)
        xTc = np.ascontiguousarray(x[bi * SEQ : (bi + 1) * SEQ, :].T)
        cols = np.concatenate(
            [
                np.arange(g * 1024, (g + 1) * 1024),
                np.arange(2048 + g * 1024, 2048 + (g + 1) * 1024),
                np.arange(4096 + g * 1024, 4096 + (g + 1) * 1024),
            ]
        )
        WTc = np.ascontiguousarray(W[cols, :].T)
        bc = np.ascontiguousarray(b[cols]).reshape(1, 3072)
        in_maps.append({"xT": xTc, "wT": WTc, "bias": bc, "biasT": bc.reshape(3072, 1)})

    res = bass_utils.run_bass_kernel_spmd(nc, in_maps, core_ids=list(range(8)))
    global LAST_RESULT
    LAST_RESULT = res

    y = np.empty((B * SEQ, E), np.float32)
    k = np.empty((B, H, SEQ, D), np.float32)
    v = np.empty((B, H, SEQ, D), np.float32)
    for c in range(8):
        bi, g = divmod(c, 2)
        r = res.results[c]
        y[bi * SEQ : (bi + 1) * SEQ, g * 1024 : (g + 1) * 1024] = r["y_out"]
        kT = r["kT_out"]
        vv = r["v_out"]
        for hh in range(NHL):
            k[bi, g * NHL + hh] = kT[hh * D : (hh + 1) * D, :].T
            v[bi, g * NHL + hh] = vv[:, hh * D : (hh + 1) * D]
    return (y, k, v)


LAST_RESULT = None
